# revision 8
# baseline (speedup 1.0000x reference)
"""Trainium2 Bass kernel for an autoregressive GRU decoder (fp16 3-pass).

Reference semantics (per row of a [B*A, .] batch, T sequential steps):
    h0 = tanh(W_lat @ lat + b_lat)
    x0 = inputs[:, :, 0, :]          # later timesteps of `inputs` are unused
    per step:
        xe = W_emb @ x + b_emb
        gx = W_ih @ xe + b_ih ; gh = W_hh @ h + b_hh
        r = sig(gxr+ghr); z = sig(gxz+ghz); n = tanh(gxn + r*ghn)
        h' = (1-z)*n + z*h
        x' = x + W_out @ h' + b_out
    output = stack of x_t, shape [B, A, T, n_in]

Device strategy (8 NeuronCores, data-parallel over B*A = 2048 rows,
256 rows per core), v2:
  - W_emb folded into W_ih on the host: W_ihe = W_ih @ W_emb [1536, 64].
  - All matmuls run as fp16 split-precision (operand = hi + lo fp16
    pair, fp32 PSUM accumulate, 3 of the 4 cross products kept). The
    recurrence is chaotic (~200x amplification of per-step error);
    CPU simulation shows 3-pass fp16 lands at 2.4e-4 final rel err
    (vs 2e-2 gate) while any 1/2-pass component fails (3e-2..8e-2).
    fp16 streams 1 row/cycle on the PE vs fp32's 4 -> 3-pass is 25%
    fewer PE cycles than fp32 AND enables x-path stacking below.
  - x-path (K=64) matmuls stack [W1;W2] along K into one K=128 matmul
    with the x operand replicated on partitions 64..127, making the
    x contribution effectively exact in 2 matmuls per gate region.
  - W_out (M=64) passes replicate the weight columns so both psum
    halves hold the full xo; x state is kept replicated [128, rows]
    so every elementwise op stays partition-aligned.
  - Per-step schedule is software-pipelined: xo_{t-1} (which needs
    h_{t-1}) is emitted interleaved with step t's r-gate matmuls so
    the PE never waits on the DVE/ACT gate tail (the fp32 baseline
    lost 7.25us/step to exactly that stall).
  - All biases are applied in exact fp32: r/z via ACT sigmoid bias
    operand (per gate region), n-gate via the two stts, b_out in the
    x-update stt. Weight fp16 tensors carry no bias rows.
  - PSUM: rp/zp/gp [128,1024] (2 banks each) + xq [128,1024] which
    holds the n-gate x contribution and, early in the next step, the
    xo accumulation (region 0) - 8 banks total.
  - x_t per step is DMAd straight to its out[t] slice; t=0 of the
    output equals x0 and is filled in on the host.
"""

import os
import sys

import numpy as np

if "/opt/trn_rl_repo" not in sys.path:
    sys.path.insert(0, "/opt/trn_rl_repo")

B, A, T = 32, 64, 128
NIN, NLAT, NEMB, NHID = 64, 64, 256, 512
NG = 3 * NHID  # 1536
NCORES = 8
R = (B * A) // NCORES  # 256 rows per core
KC = NHID // 128  # 4 hid chunks

PROFILE = False
LAST_RESULT = None  # BassKernelResults of the most recent run (for test.py)

_PROGRAM_CACHE = {}


def _build(t_steps):
    import concourse.bass as bass
    import concourse.mybir as mybir
    from concourse import tile

    F32 = mybir.dt.float32
    F16 = mybir.dt.float16
    AF = mybir.ActivationFunctionType
    OP = mybir.AluOpType

    nc = bass.Bass()

    whh1_d = nc.dram_tensor("whh1", [128, KC * NG], F16, kind="ExternalInput")
    whh2_d = nc.dram_tensor("whh2", [128, KC * NG], F16, kind="ExternalInput")
    wihs_d = nc.dram_tensor("wihs", [128, NG], F16, kind="ExternalInput")
    wou1_d = nc.dram_tensor("wou1", [128, KC * 128], F16, kind="ExternalInput")
    wou2_d = nc.dram_tensor("wou2", [128, KC * 128], F16, kind="ExternalInput")
    wlat_d = nc.dram_tensor("wlat", [NLAT + 1, NHID], F32, kind="ExternalInput")
    brz_d = nc.dram_tensor("brz", [128, 8], F32, kind="ExternalInput")
    bhhn_d = nc.dram_tensor("bhhn", [128, KC], F32, kind="ExternalInput")
    brwn_d = nc.dram_tensor("brwn", [128, KC], F32, kind="ExternalInput")
    bout_d = nc.dram_tensor("bout", [128, 1], F32, kind="ExternalInput")
    latT_d = nc.dram_tensor("latT", [NLAT + 1, R], F32, kind="ExternalInput")
    x0T_d = nc.dram_tensor("x0T", [128, R], F32, kind="ExternalInput")
    out_d = nc.dram_tensor("out", [t_steps, NIN, R], F32, kind="ExternalOutput")

    with tile.TileContext(nc) as tc:
        with (
            tc.tile_pool(name="const", bufs=1) as cpool,
            tc.tile_pool(name="state", bufs=1) as spool,
            tc.tile_pool(name="dbl", bufs=2) as dpool,
            tc.tile_pool(name="work", bufs=2) as wpool,
            tc.tile_pool(name="ps", bufs=1, space="PSUM") as ppool,
        ):
            whh1 = cpool.tile_from(whh1_d[:], name="whh1_s")
            whh2 = cpool.tile_from(whh2_d[:], name="whh2_s")
            wihs = cpool.tile_from(wihs_d[:], name="wihs_s")
            wou1 = cpool.tile_from(wou1_d[:], name="wou1_s")
            wou2 = cpool.tile_from(wou2_d[:], name="wou2_s")
            wlat = cpool.tile_from(wlat_d[:], name="wlat_s")
            brz = cpool.tile_from(brz_d[:], name="brz_s")
            bhhn = cpool.tile_from(bhhn_d[:], name="bhhn_s")
            brwn = cpool.tile_from(brwn_d[:], name="brwn_s")
            bout = cpool.tile_from(bout_d[:], name="bout_s")

            h_t = spool.tile([128, KC * R], F32, name="h_t")

            def mm(out_ap, lhsT_ap, rhs_ap, start, stop):
                nc.tensor.matmul(out_ap, lhsT_ap, rhs_ap, start=start, stop=stop)

            # lhsT slice of a whh split for gate-col g0 (0..1535), hid chunk k
            def wsl(w, k, g0):
                c = k * NG + g0
                return w[:, c : c + 128]

            def rg(j):
                return slice(j * R, (j + 1) * R)

            # region j of a (bank0, bank1) PSUM tile pair
            def reg(pair, j):
                return pair[j // 2][:, (j % 2) * R : (j % 2 + 1) * R]

            # ---- prologue: h0 = tanh(W_lat @ lat + b_lat) ----
            lat_t = wpool.tile([NLAT + 1, R], F32, tag="lat", name="lat_t")
            nc.sync.dma_start(out=lat_t[:], in_=latT_d[:])
            h0a = ppool.tile([128, 2 * R], F32, tag="gp0", name="h0a")
            h0b = ppool.tile([128, 2 * R], F32, tag="gp1", name="h0b")
            for g in range(KC):
                mm(
                    (h0a if g < 2 else h0b)[:, rg(g % 2)],
                    wlat[:, g * 128 : (g + 1) * 128],
                    lat_t[:],
                    start=(g % 2 == 0),
                    stop=(g % 2 == 1),
                )
            nc.scalar.activation(h_t[:, 0 : 2 * R], h0a[:], AF.Tanh)
            nc.scalar.activation(h_t[:, 2 * R : 4 * R], h0b[:], AF.Tanh)

            def split_h():
                h1 = dpool.tile([128, KC * R], F16, tag="h1", name="h1")
                h2 = dpool.tile([128, KC * R], F16, tag="h2", name="h2")
                for b_ in range(2):
                    sl = slice(b_ * 2 * R, (b_ + 1) * 2 * R)
                    nc.vector.tensor_copy(h1[:, sl], h_t[:, sl])
                    nc.vector.tensor_tensor(h2[:, sl], h_t[:, sl], h1[:, sl], OP.subtract)
                return h1, h2

            def split_x(xn):
                x1r = dpool.tile([128, R], F16, tag="x1r", name="x1r")
                x2r = dpool.tile([128, R], F16, tag="x2r", name="x2r")
                nc.vector.tensor_copy(x1r[:], xn[:])
                nc.vector.tensor_tensor(x2r[:], xn[:], x1r[:], OP.subtract)
                return x1r, x2r

            h1, h2 = split_h()
            xn = dpool.tile([128, R], F32, tag="xn", name="xn")
            nc.sync.dma_start(out=xn[:], in_=x0T_d[:])
            x1r, x2r = split_x(xn)

            # h-passes of one gate region j into `dst` region j, chunks `ks`
            def gate_hmm(dst, g0, ks, h1_, h2_, first_bank, stop_last=False):
                passes = ((whh1, h1_), (whh2, h1_), (whh1, h2_))
                for ki, k in enumerate(ks):
                    for pi, (w, hp) in enumerate(passes):
                        st = first_bank and ki == 0 and pi == 0
                        sp = stop_last and ki == len(ks) - 1 and pi == len(passes) - 1
                        mm(dst, wsl(w, k, g0), hp[:, rg(k)], start=st, stop=sp)

            # xo accumulation (region 0 of xq): chunks ks of W_out @ h
            def xo_mm(xq, ks, h1_, h2_, last):
                for k in ks:
                    for w, hp in ((wou1, h1_), (wou2, h1_), (wou1, h2_)):
                        st = k == 0 and w is wou1 and hp is h1_
                        sp = last and k == ks[-1] and hp is h2_
                        mm(
                            xq[:, 0:R],
                            w[:, k * 128 : (k + 1) * 128],
                            hp[:, rg(k)],
                            start=st,
                            stop=sp,
                        )

            for step in range(1, t_steps):
                pipelined = step > 1
                # ---- rp h-passes (k-chunk blocks), interleaved with the
                # previous step's xo so the PE starts each chunk as soon as
                # that chunk's h split lands ----
                rp_ = (
                    ppool.tile([128, 2 * R], F32, tag="rp0", name="rp0"),
                    ppool.tile([128, 2 * R], F32, tag="rp1", name="rp1"),
                )
                passes = ((whh1, h1), (whh2, h1), (whh1, h2))
                xo_passes = ((wou1, h1), (wou2, h1), (wou1, h2))
                for k in range(KC):
                    if pipelined:
                        for pi, (w, hp) in enumerate(xo_passes):
                            mm(
                                xq_prev0[:, 0:R],
                                w[:, k * 128 : (k + 1) * 128],
                                hp[:, rg(k)],
                                start=(k == 0 and pi == 0),
                                stop=(k == KC - 1 and pi == 2),
                            )
                    for j in range(4):
                        for pi, (w, hp) in enumerate(passes):
                            mm(
                                reg(rp_, j),
                                wsl(w, k, j * 128),
                                hp[:, rg(k)],
                                start=(k == 0 and pi == 0 and j in (0, 2)),
                                stop=False,
                            )

                if pipelined:
                    # x_{step-1} = x_{step-2} + xo + b_out  (replicated rows)
                    xn_new = dpool.tile([128, R], F32, tag="xn", name="xn")
                    nc.vector.scalar_tensor_tensor(
                        xn_new[:], xq_prev0[:, 0:R], bout[:, 0:1], xn[:], OP.add, OP.add
                    )
                    nc.sync.dma_start(out=out_d[step - 1], in_=xn_new[0:NIN, :])
                    xn = xn_new
                    x1r, x2r = split_x(xn)

                # ---- gp bank 0 first: closing it early lets the long
                # stt -> tanh tail chain start ~7us before the burst ends ----
                gp_ = (
                    ppool.tile([128, 2 * R], F32, tag="gp0", name="gp0"),
                    ppool.tile([128, 2 * R], F32, tag="gp1", name="gp1"),
                )
                for j in (0, 1):
                    gate_hmm(
                        reg(gp_, j), 1024 + j * 128, (0, 1, 2, 3), h1, h2,
                        first_bank=(j == 0), stop_last=(j == 1),
                    )

                # rp x-passes (stacked K=128: (W1+W2) @ x, both residual x parts)
                for j in range(4):
                    for xr in (x1r, x2r):
                        mm(
                            reg(rp_, j),
                            wihs[:, j * 128 : (j + 1) * 128],
                            xr[:],
                            start=False,
                            stop=(xr is x2r and j in (1, 3)),
                        )

                # ---- xq: n-gate x contribution ----
                xq_ = (
                    ppool.tile([128, 2 * R], F32, tag="xq0", name="xq0"),
                    ppool.tile([128, 2 * R], F32, tag="xq1", name="xq1"),
                )
                for j in range(4):
                    for xr in (x1r, x2r):
                        mm(
                            reg(xq_, j),
                            wihs[:, 1024 + j * 128 : 1024 + (j + 1) * 128],
                            xr[:],
                            start=(xr is x1r and j in (0, 2)),
                            stop=(xr is x2r and j in (1, 3)),
                        )

                # ---- gp bank 1 ----
                for j in (2, 3):
                    gate_hmm(
                        reg(gp_, j), 1024 + j * 128, (0, 1, 2, 3), h1, h2,
                        first_bank=(j == 2), stop_last=(j == 3),
                    )

                # ---- zp LAST: its tail chain (sig_z -> upd -> cast) is the
                # shortest, so it bounds the exposed per-step latency.
                # Bank 0 (regions 0,1) completes x-passes included BEFORE
                # bank 1 starts, so sig_z/upd/cast for chunks 0,1 overlap
                # the bank-1 matmuls instead of being exposed at the end ----
                zp_ = (
                    ppool.tile([128, 2 * R], F32, tag="zp0", name="zp0"),
                    ppool.tile([128, 2 * R], F32, tag="zp1", name="zp1"),
                )
                for bb in range(2):
                    js = (2 * bb, 2 * bb + 1)
                    for j in js:
                        gate_hmm(reg(zp_, j), 512 + j * 128, (0, 1, 2, 3), h1, h2, first_bank=(j == js[0]))
                    for j in js:
                        for xr in (x1r, x2r):
                            mm(
                                reg(zp_, j),
                                wihs[:, 512 + j * 128 : 512 + (j + 1) * 128],
                                xr[:],
                                start=False,
                                stop=(xr is x2r and j == js[1]),
                            )

                # ---- gate tail (region-granular pipeline) ----
                r_t = wpool.tile([128, KC * R], F32, tag="r", name="r_t")
                for j in range(4):
                    nc.scalar.activation(
                        r_t[:, rg(j)], reg(rp_, j), AF.Sigmoid, bias=brz[:, j : j + 1]
                    )

                u_t = wpool.tile([128, KC * R], F32, tag="u", name="u_t")
                n_t = wpool.tile([128, KC * R], F32, tag="n", name="n_t")
                for j in range(4):
                    # u = (ghn + b_hh[n]) * r
                    nc.vector.scalar_tensor_tensor(
                        u_t[:, rg(j)], reg(gp_, j), bhhn[:, j : j + 1], r_t[:, rg(j)],
                        OP.add, OP.mult,
                    )
                    # xq = (gxn + b_row[n]) + u   (in place, PSUM)
                    nc.vector.scalar_tensor_tensor(
                        reg(xq_, j), reg(xq_, j), brwn[:, j : j + 1], u_t[:, rg(j)],
                        OP.add, OP.add,
                    )
                for j in range(4):
                    nc.scalar.activation(n_t[:, rg(j)], reg(xq_, j), AF.Tanh)

                z_t = wpool.tile([128, KC * R], F32, tag="z", name="z_t")
                for j in range(4):
                    nc.scalar.activation(
                        z_t[:, rg(j)], reg(zp_, j), AF.Sigmoid, bias=brz[:, 4 + j : 5 + j]
                    )

                h1n = dpool.tile([128, KC * R], F16, tag="h1", name="h1")
                h2n = dpool.tile([128, KC * R], F16, tag="h2", name="h2")
                for j in range(4):
                    sl = rg(j)
                    # h' = n + z*(h - n); the fp16 h1 copy is produced
                    # directly by a second add so the next step's matmuls
                    # don't wait for a separate cast
                    nc.vector.tensor_tensor(h_t[:, sl], h_t[:, sl], n_t[:, sl], OP.subtract)
                    nc.vector.tensor_tensor(h_t[:, sl], z_t[:, sl], h_t[:, sl], OP.mult)
                    nc.vector.tensor_tensor(h1n[:, sl], n_t[:, sl], h_t[:, sl], OP.add)
                    nc.vector.tensor_tensor(h_t[:, sl], n_t[:, sl], h_t[:, sl], OP.add)
                    nc.vector.tensor_tensor(h2n[:, sl], h_t[:, sl], h1n[:, sl], OP.subtract)
                h1, h2 = h1n, h2n
                xq_prev0 = xq_[0]

            # ---- epilogue: last xo / x update ----
            xo_mm(xq_prev0, (0, 1, 2, 3), h1, h2, last=True)
            xn_new = dpool.tile([128, R], F32, tag="xn", name="xn")
            nc.vector.scalar_tensor_tensor(
                xn_new[:], xq_prev0[:, 0:R], bout[:, 0:1], xn[:], OP.add, OP.add
            )
            nc.sync.dma_start(out=out_d[t_steps - 1], in_=xn_new[0:NIN, :])

    return nc


def _fix_wait_overflow(nc):
    """Split semaphore waits that exceed per-instruction ISA capacity.

    walrus rejects engine instructions with >1 sync wait (and DMAs with
    >2). Excess waits move to a same-engine InstDrain inserted
    immediately before the instruction - the engine is in-order, so the
    stall point is unchanged. (Tile's own kernel-tail drains carry 10+
    waits, so drains have no such capacity limit.)
    """
    import concourse.mybir as mybir

    caps = {"InstMatmult": 1, "InstDMACopy": 1, "InstTensorScalarPtr": 1,
            "InstTensorTensor": 1, "InstActivation": 1, "InstMemset": 1,
            "InstTensorCopy": 1, "InstTensorScalar": 1, "InstDrain": 1}
    for f in nc.m.functions:
        for blk in f.blocks:
            insts = list(blk.instructions)
            out = []
            changed = False
            for inst in insts:
                si = inst.sync_info
                ow = list(si.on_wait) if si and si.on_wait else []
                cap = caps.get(type(inst).__name__)
                if cap is not None and len(ow) > cap:
                    excess = ow[cap:]
                    dcap = caps["InstDrain"]
                    for i in range(0, len(excess), dcap):
                        d = mybir.InstDrain(
                            name=nc.get_next_instruction_name(),
                            ins=[],
                            outs=[],
                            bass_is_fusable=False,
                        )
                        d.engine = inst.engine
                        d.sync_info = mybir.SyncInfo(
                            on_wait=excess[i : i + dcap], on_update=[]
                        )
                        out.append(d)
                    inst.sync_info = mybir.SyncInfo(
                        on_wait=ow[:cap],
                        on_update=list(si.on_update) if si.on_update else [],
                    )
                    changed = True
                out.append(inst)
            if changed:
                blk.instructions = out
    return nc


def _get_program(t_steps):
    if t_steps not in _PROGRAM_CACHE:
        _PROGRAM_CACHE[t_steps] = _fix_wait_overflow(_build(t_steps))
    return _PROGRAM_CACHE[t_steps]


def _split16(a):
    a = np.asarray(a, np.float32)
    hi = a.astype(np.float16)
    lo = (a - hi.astype(np.float32)).astype(np.float16)
    return hi, lo


def _host_prep(latents, inputs, W_lat, b_lat, W_emb, b_emb, W_out, b_out, W_ih, b_ih, W_hh, b_hh):
    f32 = np.float32
    f64 = np.float64
    lat = np.asarray(latents, f32).reshape(B * A, NLAT)
    x0 = np.ascontiguousarray(np.asarray(inputs, f32)[:, :, 0, :]).reshape(B * A, NIN)

    W_ih64 = np.asarray(W_ih, f64)
    W_ihe = (W_ih64 @ np.asarray(W_emb, f64)).astype(f32)  # [1536, 64]
    b_row = (W_ih64 @ np.asarray(b_emb, f64) + np.asarray(b_ih, f64)).astype(f32)
    b_hh32 = np.asarray(b_hh, f32)

    whh = np.ascontiguousarray(
        np.asarray(W_hh, f32).T.reshape(KC, 128, NG).transpose(1, 0, 2).reshape(128, KC * NG)
    )
    whh1, whh2 = _split16(whh)

    w1, w2 = _split16(W_ihe)
    wihs = np.ascontiguousarray(np.concatenate([w1.T, w2.T], axis=0))  # [128, 1536]

    w1o, w2o = _split16(W_out)  # [64, 512] each
    wou1 = np.ascontiguousarray(
        np.concatenate(
            [np.concatenate([w1o[:, k * 128 : (k + 1) * 128].T] * 2, axis=1) for k in range(KC)],
            axis=1,
        )
    )  # [128, KC*128]
    wou2 = np.ascontiguousarray(
        np.concatenate(
            [np.concatenate([w2o[:, k * 128 : (k + 1) * 128].T] * 2, axis=1) for k in range(KC)],
            axis=1,
        )
    )

    wlat = np.empty((NLAT + 1, NHID), f32)
    wlat[:NLAT] = np.asarray(W_lat, f32).T
    wlat[NLAT] = np.asarray(b_lat, f32)

    brz_full = (b_row + b_hh32)[: 2 * NHID]
    brz = np.empty((128, 8), f32)
    for j in range(4):
        brz[:, j] = brz_full[j * 128 : (j + 1) * 128]
        brz[:, 4 + j] = brz_full[NHID + j * 128 : NHID + (j + 1) * 128]
    bhhn = np.ascontiguousarray(b_hh32[2 * NHID :].reshape(KC, 128).T)
    brwn = np.ascontiguousarray(b_row[2 * NHID :].reshape(KC, 128).T)
    bout2 = np.ascontiguousarray(np.tile(np.asarray(b_out, f32), 2)[:, None])  # [128,1]

    shared = dict(
        whh1=whh1, whh2=whh2, wihs=wihs, wou1=wou1, wou2=wou2, wlat=wlat,
        brz=brz, bhhn=bhhn, brwn=brwn, bout=bout2,
    )
    in_maps = []
    for c in range(NCORES):
        sl = slice(c * R, (c + 1) * R)
        latT = np.empty((NLAT + 1, R), f32)
        latT[:NLAT] = lat[sl].T
        latT[NLAT] = 1.0
        x0T = np.ascontiguousarray(np.concatenate([x0[sl].T] * 2, axis=0))  # [128, R]
        in_maps.append(dict(shared, latT=latT, x0T=x0T))
    return in_maps


def kernel(**inputs):
    global LAST_RESULT
    from concourse import bass_utils

    in_maps = _host_prep(**inputs)
    nc = _get_program(T)
    kwargs = {}
    if PROFILE:
        kwargs = dict(trace=True, trace_cores=[0])
    res = bass_utils.run_bass_kernel_spmd(nc, in_maps, list(range(NCORES)), **kwargs)
    LAST_RESULT = res

    # per-core out is [T, NIN, R] -> rows-major [R, T, NIN]
    parts = [res.results[c]["out"].transpose(2, 0, 1) for c in range(NCORES)]
    full = np.concatenate(parts, axis=0)  # [B*A, T, NIN]
    out = full.reshape(B, A, T, NIN).astype(np.float32, copy=True)
    # the device never writes slot t=0; it is exactly x0
    out[:, :, 0, :] = np.asarray(inputs["inputs"], np.float32)[:, :, 0, :]
    return out


# revision 9
# speedup vs baseline: 1.0013x; 1.0013x over previous
"""Trainium2 Bass kernel for an autoregressive GRU decoder (fp16 3-pass).

Reference semantics (per row of a [B*A, .] batch, T sequential steps):
    h0 = tanh(W_lat @ lat + b_lat)
    x0 = inputs[:, :, 0, :]          # later timesteps of `inputs` are unused
    per step:
        xe = W_emb @ x + b_emb
        gx = W_ih @ xe + b_ih ; gh = W_hh @ h + b_hh
        r = sig(gxr+ghr); z = sig(gxz+ghz); n = tanh(gxn + r*ghn)
        h' = (1-z)*n + z*h
        x' = x + W_out @ h' + b_out
    output = stack of x_t, shape [B, A, T, n_in]

Device strategy (8 NeuronCores, data-parallel over B*A = 2048 rows,
256 rows per core), v2:
  - W_emb folded into W_ih on the host: W_ihe = W_ih @ W_emb [1536, 64].
  - All matmuls run as fp16 split-precision (operand = hi + lo fp16
    pair, fp32 PSUM accumulate, 3 of the 4 cross products kept). The
    recurrence is chaotic (~200x amplification of per-step error);
    CPU simulation shows 3-pass fp16 lands at 2.4e-4 final rel err
    (vs 2e-2 gate) while any 1/2-pass component fails (3e-2..8e-2).
    fp16 streams 1 row/cycle on the PE vs fp32's 4 -> 3-pass is 25%
    fewer PE cycles than fp32 AND enables x-path stacking below.
  - x-path (K=64) matmuls stack [W1;W2] along K into one K=128 matmul
    with the x operand replicated on partitions 64..127, making the
    x contribution effectively exact in 2 matmuls per gate region.
  - W_out (M=64) passes replicate the weight columns so both psum
    halves hold the full xo; x state is kept replicated [128, rows]
    so every elementwise op stays partition-aligned.
  - Per-step schedule is software-pipelined: xo_{t-1} (which needs
    h_{t-1}) is emitted interleaved with step t's r-gate matmuls so
    the PE never waits on the DVE/ACT gate tail (the fp32 baseline
    lost 7.25us/step to exactly that stall).
  - All biases are applied in exact fp32: r/z via ACT sigmoid bias
    operand (per gate region), n-gate via the two stts, b_out in the
    x-update stt. Weight fp16 tensors carry no bias rows.
  - PSUM: rp/zp/gp [128,1024] (2 banks each) + xq [128,1024] which
    holds the n-gate x contribution and, early in the next step, the
    xo accumulation (region 0) - 8 banks total.
  - x_t per step is DMAd straight to its out[t] slice; t=0 of the
    output equals x0 and is filled in on the host.
"""

import os
import sys

import numpy as np

if "/opt/trn_rl_repo" not in sys.path:
    sys.path.insert(0, "/opt/trn_rl_repo")

B, A, T = 32, 64, 128
NIN, NLAT, NEMB, NHID = 64, 64, 256, 512
NG = 3 * NHID  # 1536
NCORES = 8
R = (B * A) // NCORES  # 256 rows per core
KC = NHID // 128  # 4 hid chunks

PROFILE = False
LAST_RESULT = None  # BassKernelResults of the most recent run (for test.py)

_PROGRAM_CACHE = {}


def _build(t_steps):
    import concourse.bass as bass
    import concourse.mybir as mybir
    from concourse import tile

    F32 = mybir.dt.float32
    F16 = mybir.dt.float16
    AF = mybir.ActivationFunctionType
    OP = mybir.AluOpType

    nc = bass.Bass()

    whh1_d = nc.dram_tensor("whh1", [128, KC * NG], F16, kind="ExternalInput")
    whh2_d = nc.dram_tensor("whh2", [128, KC * NG], F16, kind="ExternalInput")
    wihs_d = nc.dram_tensor("wihs", [128, NG], F16, kind="ExternalInput")
    wou1_d = nc.dram_tensor("wou1", [128, KC * 128], F16, kind="ExternalInput")
    wou2_d = nc.dram_tensor("wou2", [128, KC * 128], F16, kind="ExternalInput")
    wlat_d = nc.dram_tensor("wlat", [NLAT + 1, NHID], F32, kind="ExternalInput")
    brz_d = nc.dram_tensor("brz", [128, 8], F32, kind="ExternalInput")
    bhhn_d = nc.dram_tensor("bhhn", [128, KC], F32, kind="ExternalInput")
    brwn_d = nc.dram_tensor("brwn", [128, KC], F32, kind="ExternalInput")
    bout_d = nc.dram_tensor("bout", [128, 1], F32, kind="ExternalInput")
    latT_d = nc.dram_tensor("latT", [NLAT + 1, R], F32, kind="ExternalInput")
    x0T_d = nc.dram_tensor("x0T", [128, R], F32, kind="ExternalInput")
    out_d = nc.dram_tensor("out", [t_steps, NIN, R], F32, kind="ExternalOutput")

    with tile.TileContext(nc) as tc:
        with (
            tc.tile_pool(name="const", bufs=1) as cpool,
            tc.tile_pool(name="state", bufs=1) as spool,
            tc.tile_pool(name="dbl", bufs=2) as dpool,
            tc.tile_pool(name="work", bufs=2) as wpool,
            tc.tile_pool(name="ps", bufs=1, space="PSUM") as ppool,
        ):
            whh1 = cpool.tile_from(whh1_d[:], name="whh1_s")
            whh2 = cpool.tile_from(whh2_d[:], name="whh2_s")
            wihs = cpool.tile_from(wihs_d[:], name="wihs_s")
            wou1 = cpool.tile_from(wou1_d[:], name="wou1_s")
            wou2 = cpool.tile_from(wou2_d[:], name="wou2_s")
            wlat = cpool.tile_from(wlat_d[:], name="wlat_s")
            brz = cpool.tile_from(brz_d[:], name="brz_s")
            bhhn = cpool.tile_from(bhhn_d[:], name="bhhn_s")
            brwn = cpool.tile_from(brwn_d[:], name="brwn_s")
            bout = cpool.tile_from(bout_d[:], name="bout_s")

            h_t = spool.tile([128, KC * R], F32, name="h_t")

            def mm(out_ap, lhsT_ap, rhs_ap, start, stop):
                nc.tensor.matmul(out_ap, lhsT_ap, rhs_ap, start=start, stop=stop)

            # lhsT slice of a whh split for gate-col g0 (0..1535), hid chunk k
            def wsl(w, k, g0):
                c = k * NG + g0
                return w[:, c : c + 128]

            def rg(j):
                return slice(j * R, (j + 1) * R)

            # region j of a (bank0, bank1) PSUM tile pair
            def reg(pair, j):
                return pair[j // 2][:, (j % 2) * R : (j % 2 + 1) * R]

            # ---- prologue: h0 = tanh(W_lat @ lat + b_lat) ----
            lat_t = wpool.tile([NLAT + 1, R], F32, tag="lat", name="lat_t")
            nc.sync.dma_start(out=lat_t[:], in_=latT_d[:])
            h0a = ppool.tile([128, 2 * R], F32, tag="gp0", name="h0a")
            h0b = ppool.tile([128, 2 * R], F32, tag="gp1", name="h0b")
            for g in range(KC):
                mm(
                    (h0a if g < 2 else h0b)[:, rg(g % 2)],
                    wlat[:, g * 128 : (g + 1) * 128],
                    lat_t[:],
                    start=(g % 2 == 0),
                    stop=(g % 2 == 1),
                )
            nc.scalar.activation(h_t[:, 0 : 2 * R], h0a[:], AF.Tanh)
            nc.scalar.activation(h_t[:, 2 * R : 4 * R], h0b[:], AF.Tanh)

            def split_h():
                h1 = dpool.tile([128, KC * R], F16, tag="h1", name="h1")
                h2 = dpool.tile([128, KC * R], F16, tag="h2", name="h2")
                for b_ in range(2):
                    sl = slice(b_ * 2 * R, (b_ + 1) * 2 * R)
                    nc.vector.tensor_copy(h1[:, sl], h_t[:, sl])
                    nc.vector.tensor_tensor(h2[:, sl], h_t[:, sl], h1[:, sl], OP.subtract)
                return h1, h2

            def split_x(xn):
                x1r = dpool.tile([128, R], F16, tag="x1r", name="x1r")
                x2r = dpool.tile([128, R], F16, tag="x2r", name="x2r")
                nc.vector.tensor_copy(x1r[:], xn[:])
                nc.vector.tensor_tensor(x2r[:], xn[:], x1r[:], OP.subtract)
                return x1r, x2r

            h1, h2 = split_h()
            xn = dpool.tile([128, R], F32, tag="xn", name="xn")
            nc.sync.dma_start(out=xn[:], in_=x0T_d[:])
            x1r, x2r = split_x(xn)

            # h-passes of one gate region j into `dst` region j, chunks `ks`
            def gate_hmm(dst, g0, ks, h1_, h2_, first_bank, stop_last=False):
                passes = ((whh1, h1_), (whh2, h1_), (whh1, h2_))
                for ki, k in enumerate(ks):
                    for pi, (w, hp) in enumerate(passes):
                        st = first_bank and ki == 0 and pi == 0
                        sp = stop_last and ki == len(ks) - 1 and pi == len(passes) - 1
                        mm(dst, wsl(w, k, g0), hp[:, rg(k)], start=st, stop=sp)

            # xo accumulation (region 0 of xq): chunks ks of W_out @ h
            def xo_mm(xq, ks, h1_, h2_, last):
                for k in ks:
                    for w, hp in ((wou1, h1_), (wou2, h1_), (wou1, h2_)):
                        st = k == 0 and w is wou1 and hp is h1_
                        sp = last and k == ks[-1] and hp is h2_
                        mm(
                            xq[:, 0:R],
                            w[:, k * 128 : (k + 1) * 128],
                            hp[:, rg(k)],
                            start=st,
                            stop=sp,
                        )

            for step in range(1, t_steps):
                pipelined = step > 1
                # ---- rp h-passes (k-chunk blocks), interleaved with the
                # previous step's xo so the PE starts each chunk as soon as
                # that chunk's h split lands ----
                rp_ = (
                    ppool.tile([128, 2 * R], F32, tag="rp0", name="rp0"),
                    ppool.tile([128, 2 * R], F32, tag="rp1", name="rp1"),
                )
                passes = ((whh1, h1), (whh2, h1), (whh1, h2))
                xo_passes = ((wou1, h1), (wou2, h1), (wou1, h2))
                for k in range(KC):
                    if pipelined:
                        for pi, (w, hp) in enumerate(xo_passes):
                            mm(
                                xq_prev0[:, 0:R],
                                w[:, k * 128 : (k + 1) * 128],
                                hp[:, rg(k)],
                                start=(k == 0 and pi == 0),
                                stop=(k == KC - 1 and pi == 2),
                            )
                    for j in range(4):
                        for pi, (w, hp) in enumerate(passes):
                            mm(
                                reg(rp_, j),
                                wsl(w, k, j * 128),
                                hp[:, rg(k)],
                                start=(k == 0 and pi == 0 and j in (0, 2)),
                                stop=False,
                            )

                if pipelined:
                    # x_{step-1} = x_{step-2} + xo + b_out  (replicated rows)
                    xn_new = dpool.tile([128, R], F32, tag="xn", name="xn")
                    nc.vector.scalar_tensor_tensor(
                        xn_new[:], xq_prev0[:, 0:R], bout[:, 0:1], xn[:], OP.add, OP.add
                    )
                    nc.sync.dma_start(out=out_d[step - 1], in_=xn_new[0:NIN, :])
                    xn = xn_new
                    x1r, x2r = split_x(xn)

                # ---- gp bank 0 first: closing it early lets the long
                # stt -> tanh tail chain start ~7us before the burst ends ----
                gp_ = (
                    ppool.tile([128, 2 * R], F32, tag="gp0", name="gp0"),
                    ppool.tile([128, 2 * R], F32, tag="gp1", name="gp1"),
                )
                for j in (0, 1):
                    gate_hmm(
                        reg(gp_, j), 1024 + j * 128, (0, 1, 2, 3), h1, h2,
                        first_bank=(j == 0), stop_last=(j == 1),
                    )

                # rp x-passes (stacked K=128: (W1+W2) @ x, both residual x parts)
                for j in range(4):
                    for xr in (x1r, x2r):
                        mm(
                            reg(rp_, j),
                            wihs[:, j * 128 : (j + 1) * 128],
                            xr[:],
                            start=False,
                            stop=(xr is x2r and j in (1, 3)),
                        )

                # ---- xq: n-gate x contribution ----
                xq_ = (
                    ppool.tile([128, 2 * R], F32, tag="xq0", name="xq0"),
                    ppool.tile([128, 2 * R], F32, tag="xq1", name="xq1"),
                )
                for j in range(4):
                    for xr in (x1r, x2r):
                        mm(
                            reg(xq_, j),
                            wihs[:, 1024 + j * 128 : 1024 + (j + 1) * 128],
                            xr[:],
                            start=(xr is x1r and j in (0, 2)),
                            stop=(xr is x2r and j in (1, 3)),
                        )

                # ---- zp LAST: its tail chain (sig_z -> upd -> cast) is the
                # shortest, so it bounds the exposed per-step latency.
                # Bank 0 (regions 0,1) completes x-passes included BEFORE
                # bank 1 starts, so sig_z/upd/cast for chunks 0,1 overlap
                # the bank-1 matmuls instead of being exposed at the end ----
                zp_ = (
                    ppool.tile([128, 2 * R], F32, tag="zp0", name="zp0"),
                    ppool.tile([128, 2 * R], F32, tag="zp1", name="zp1"),
                )

                def zp_bank(bb):
                    js = (2 * bb, 2 * bb + 1)
                    for j in js:
                        gate_hmm(reg(zp_, j), 512 + j * 128, (0, 1, 2, 3), h1, h2, first_bank=(j == js[0]))
                    for j in js:
                        for xr in (x1r, x2r):
                            mm(
                                reg(zp_, j),
                                wihs[:, 512 + j * 128 : 512 + (j + 1) * 128],
                                xr[:],
                                start=False,
                                stop=(xr is x2r and j == js[1]),
                            )

                # zp bank0 early: its stop releases the bank-0 tail chain
                # ~5us before the burst ends
                zp_bank(0)
                # gp bank 1
                for j in (2, 3):
                    gate_hmm(
                        reg(gp_, j), 1024 + j * 128, (0, 1, 2, 3), h1, h2,
                        first_bank=(j == 2), stop_last=(j == 3),
                    )
                zp_bank(1)

                # ---- gate tail (region-granular pipeline) ----
                r_t = wpool.tile([128, KC * R], F32, tag="r", name="r_t")
                for j in range(4):
                    nc.scalar.activation(
                        r_t[:, rg(j)], reg(rp_, j), AF.Sigmoid, bias=brz[:, j : j + 1]
                    )

                u_t = wpool.tile([128, KC * R], F32, tag="u", name="u_t")
                n_t = wpool.tile([128, KC * R], F32, tag="n", name="n_t")
                z_t = wpool.tile([128, KC * R], F32, tag="z", name="z_t")
                h1n = dpool.tile([128, KC * R], F16, tag="h1", name="h1")
                h2n = dpool.tile([128, KC * R], F16, tag="h2", name="h2")
                for bb in range(2):
                    js = (2 * bb, 2 * bb + 1)
                    for j in js:
                        # u = (ghn + b_hh[n]) * r
                        nc.vector.scalar_tensor_tensor(
                            u_t[:, rg(j)], reg(gp_, j), bhhn[:, j : j + 1], r_t[:, rg(j)],
                            OP.add, OP.mult,
                        )
                        # xq = (gxn + b_row[n]) + u   (in place, PSUM)
                        nc.vector.scalar_tensor_tensor(
                            reg(xq_, j), reg(xq_, j), brwn[:, j : j + 1], u_t[:, rg(j)],
                            OP.add, OP.add,
                        )
                    for j in js:
                        nc.scalar.activation(n_t[:, rg(j)], reg(xq_, j), AF.Tanh)
                    for j in js:
                        nc.scalar.activation(
                            z_t[:, rg(j)], reg(zp_, j), AF.Sigmoid, bias=brz[:, 4 + j : 5 + j]
                        )
                    for j in js:
                        sl = rg(j)
                        # h' = n + z*(h - n); the fp16 h1 copy is produced
                        # directly by a second add so the next step's
                        # matmuls don't wait for a separate cast
                        nc.vector.tensor_tensor(h_t[:, sl], h_t[:, sl], n_t[:, sl], OP.subtract)
                        nc.vector.tensor_tensor(h_t[:, sl], z_t[:, sl], h_t[:, sl], OP.mult)
                        nc.vector.tensor_tensor(h1n[:, sl], n_t[:, sl], h_t[:, sl], OP.add)
                        nc.vector.tensor_tensor(h_t[:, sl], n_t[:, sl], h_t[:, sl], OP.add)
                        nc.vector.tensor_tensor(h2n[:, sl], h_t[:, sl], h1n[:, sl], OP.subtract)
                h1, h2 = h1n, h2n
                xq_prev0 = xq_[0]

            # ---- epilogue: last xo / x update ----
            xo_mm(xq_prev0, (0, 1, 2, 3), h1, h2, last=True)
            xn_new = dpool.tile([128, R], F32, tag="xn", name="xn")
            nc.vector.scalar_tensor_tensor(
                xn_new[:], xq_prev0[:, 0:R], bout[:, 0:1], xn[:], OP.add, OP.add
            )
            nc.sync.dma_start(out=out_d[t_steps - 1], in_=xn_new[0:NIN, :])

    return nc


def _fix_wait_overflow(nc):
    """Split semaphore waits that exceed per-instruction ISA capacity.

    walrus rejects engine instructions with >1 sync wait (and DMAs with
    >2). Excess waits move to a same-engine InstDrain inserted
    immediately before the instruction - the engine is in-order, so the
    stall point is unchanged. (Tile's own kernel-tail drains carry 10+
    waits, so drains have no such capacity limit.)
    """
    import concourse.mybir as mybir

    caps = {"InstMatmult": 1, "InstDMACopy": 1, "InstTensorScalarPtr": 1,
            "InstTensorTensor": 1, "InstActivation": 1, "InstMemset": 1,
            "InstTensorCopy": 1, "InstTensorScalar": 1, "InstDrain": 1}
    for f in nc.m.functions:
        for blk in f.blocks:
            insts = list(blk.instructions)
            out = []
            changed = False
            for inst in insts:
                si = inst.sync_info
                ow = list(si.on_wait) if si and si.on_wait else []
                cap = caps.get(type(inst).__name__)
                if cap is not None and len(ow) > cap:
                    excess = ow[cap:]
                    dcap = caps["InstDrain"]
                    for i in range(0, len(excess), dcap):
                        d = mybir.InstDrain(
                            name=nc.get_next_instruction_name(),
                            ins=[],
                            outs=[],
                            bass_is_fusable=False,
                        )
                        d.engine = inst.engine
                        d.sync_info = mybir.SyncInfo(
                            on_wait=excess[i : i + dcap], on_update=[]
                        )
                        out.append(d)
                    inst.sync_info = mybir.SyncInfo(
                        on_wait=ow[:cap],
                        on_update=list(si.on_update) if si.on_update else [],
                    )
                    changed = True
                out.append(inst)
            if changed:
                blk.instructions = out
    return nc


def _get_program(t_steps):
    if t_steps not in _PROGRAM_CACHE:
        _PROGRAM_CACHE[t_steps] = _fix_wait_overflow(_build(t_steps))
    return _PROGRAM_CACHE[t_steps]


def _split16(a):
    a = np.asarray(a, np.float32)
    hi = a.astype(np.float16)
    lo = (a - hi.astype(np.float32)).astype(np.float16)
    return hi, lo


def _host_prep(latents, inputs, W_lat, b_lat, W_emb, b_emb, W_out, b_out, W_ih, b_ih, W_hh, b_hh):
    f32 = np.float32
    f64 = np.float64
    lat = np.asarray(latents, f32).reshape(B * A, NLAT)
    x0 = np.ascontiguousarray(np.asarray(inputs, f32)[:, :, 0, :]).reshape(B * A, NIN)

    W_ih64 = np.asarray(W_ih, f64)
    W_ihe = (W_ih64 @ np.asarray(W_emb, f64)).astype(f32)  # [1536, 64]
    b_row = (W_ih64 @ np.asarray(b_emb, f64) + np.asarray(b_ih, f64)).astype(f32)
    b_hh32 = np.asarray(b_hh, f32)

    whh = np.ascontiguousarray(
        np.asarray(W_hh, f32).T.reshape(KC, 128, NG).transpose(1, 0, 2).reshape(128, KC * NG)
    )
    whh1, whh2 = _split16(whh)

    w1, w2 = _split16(W_ihe)
    wihs = np.ascontiguousarray(np.concatenate([w1.T, w2.T], axis=0))  # [128, 1536]

    w1o, w2o = _split16(W_out)  # [64, 512] each
    wou1 = np.ascontiguousarray(
        np.concatenate(
            [np.concatenate([w1o[:, k * 128 : (k + 1) * 128].T] * 2, axis=1) for k in range(KC)],
            axis=1,
        )
    )  # [128, KC*128]
    wou2 = np.ascontiguousarray(
        np.concatenate(
            [np.concatenate([w2o[:, k * 128 : (k + 1) * 128].T] * 2, axis=1) for k in range(KC)],
            axis=1,
        )
    )

    wlat = np.empty((NLAT + 1, NHID), f32)
    wlat[:NLAT] = np.asarray(W_lat, f32).T
    wlat[NLAT] = np.asarray(b_lat, f32)

    brz_full = (b_row + b_hh32)[: 2 * NHID]
    brz = np.empty((128, 8), f32)
    for j in range(4):
        brz[:, j] = brz_full[j * 128 : (j + 1) * 128]
        brz[:, 4 + j] = brz_full[NHID + j * 128 : NHID + (j + 1) * 128]
    bhhn = np.ascontiguousarray(b_hh32[2 * NHID :].reshape(KC, 128).T)
    brwn = np.ascontiguousarray(b_row[2 * NHID :].reshape(KC, 128).T)
    bout2 = np.ascontiguousarray(np.tile(np.asarray(b_out, f32), 2)[:, None])  # [128,1]

    shared = dict(
        whh1=whh1, whh2=whh2, wihs=wihs, wou1=wou1, wou2=wou2, wlat=wlat,
        brz=brz, bhhn=bhhn, brwn=brwn, bout=bout2,
    )
    in_maps = []
    for c in range(NCORES):
        sl = slice(c * R, (c + 1) * R)
        latT = np.empty((NLAT + 1, R), f32)
        latT[:NLAT] = lat[sl].T
        latT[NLAT] = 1.0
        x0T = np.ascontiguousarray(np.concatenate([x0[sl].T] * 2, axis=0))  # [128, R]
        in_maps.append(dict(shared, latT=latT, x0T=x0T))
    return in_maps


def kernel(**inputs):
    global LAST_RESULT
    from concourse import bass_utils

    in_maps = _host_prep(**inputs)
    nc = _get_program(T)
    kwargs = {}
    if PROFILE:
        kwargs = dict(trace=True, trace_cores=[0])
    res = bass_utils.run_bass_kernel_spmd(nc, in_maps, list(range(NCORES)), **kwargs)
    LAST_RESULT = res

    # per-core out is [T, NIN, R] -> rows-major [R, T, NIN]
    parts = [res.results[c]["out"].transpose(2, 0, 1) for c in range(NCORES)]
    full = np.concatenate(parts, axis=0)  # [B*A, T, NIN]
    out = full.reshape(B, A, T, NIN).astype(np.float32, copy=True)
    # the device never writes slot t=0; it is exactly x0
    out[:, :, 0, :] = np.asarray(inputs["inputs"], np.float32)[:, :, 0, :]
    return out
